# revision 9
# baseline (speedup 1.0000x reference)
"""Size-weighted focal loss on 8 Trainium2 NeuronCores — v5.

Math (per element, x = logit, t in {0,1}):
  w  = x*(1-2t)         so (1-pt) = sigmoid(w) = u
  L  = -log(pt) = softplus(w) = -ln(1-u)
  a  = 0.75 - 0.5*t     (alpha_t with ALPHA=0.25)
  elem = a * u^2 * L

Host packs w = bf16_rne(x) sign-flipped by t, with t stowed in the bf16
LSB (<=1ulp perturbation). Device input is 4MB/core instead of 16MB —
the baseline's DMA bottleneck — and the strided hi16-XOR DVE pass
disappears.

Device (per core, 8 samples, phase-ordered so each ACT table loads once):
  u    = Sigmoid(w)            [ACT pass 1, table sigmoid_and_others]
  Lv   = Ln(1 - u)  = -L       [ACT pass 2, table natural_log]
  tm   = (w&1) - 1.5 = t - 1.5 [DVE ts, int AND then float SUB]
  g    = tm * u                [DVE tt]
  F    = g * u = (t-1.5)*u^2   [DVE tt]
  PE per sample: psum[128,128] += Lv_chunk^T @ F_chunk  (16 chunks)
  diag extract with mask M[i,i] = 0.5:
    Scol[:,b] = 0.5*sum_diag = sum(a*u^2*L) partials per partition-slot
  (elem = a*u^2*L = 0.5*F*Lv since a = -0.5*(t-1.5), L = -Lv)

Host: fg_b = count_nonzero(target_b); mean_b( (S_b/HW) * sw(fg_b) ).
"""

import numpy as np
from contextlib import ExitStack

P = 128
B_PER_CORE = 8
N_CORES = 8
H = 512
W = 512
HW = H * W                 # 262144
FREE = HW // P             # 2048 per sample
NCHUNK = FREE // P         # 16 chunks per sample

_GLOBAL = {}


def _build():
    import concourse.bacc as bacc
    import concourse.tile as tile
    import concourse.mybir as mybir

    f32 = mybir.dt.float32
    bf16 = mybir.dt.bfloat16
    u16 = mybir.dt.uint16
    Alu = mybir.AluOpType
    Act = mybir.ActivationFunctionType

    nc = bacc.Bacc("TRN2", target_bir_lowering=False, debug=False,
                   num_devices=N_CORES)

    wp_in = nc.dram_tensor("wp", (P, B_PER_CORE, FREE), bf16, kind="ExternalInput")
    mask_in = nc.dram_tensor("mask", (P, P), f32, kind="ExternalInput")
    out_t = nc.dram_tensor("out", (P, B_PER_CORE), f32, kind="ExternalOutput")

    w_v = wp_in.ap()

    with ExitStack() as ctx:
        tc = ctx.enter_context(tile.TileContext(nc))
        singles = ctx.enter_context(tc.tile_pool(name="singles", bufs=1))
        u2pool = ctx.enter_context(tc.tile_pool(name="u2pool", bufs=3))
        s2pool = ctx.enter_context(tc.tile_pool(name="s2pool", bufs=2))
        fpool = ctx.enter_context(tc.tile_pool(name="fpool", bufs=8))
        lvpool = ctx.enter_context(tc.tile_pool(name="lvpool", bufs=3))
        scrpool = ctx.enter_context(tc.tile_pool(name="scrpool", bufs=2))
        psum = ctx.enter_context(tc.tile_pool(name="psum", bufs=8, space="PSUM"))

        mask_t = singles.tile([P, P], f32)
        shift15_t = singles.tile([P, 1], u16)
        nc.vector.memset(shift15_t[:], 15)
        Scol = singles.tile([P, B_PER_CORE], f32)
        wt = singles.tile([P, B_PER_CORE * FREE], bf16)   # packed w, all samples
        ut = singles.tile([P, B_PER_CORE * FREE], bf16)   # sigmoid(w)

        def sl(b):
            return slice(b * FREE, (b + 1) * FREE)

        # DMA: few calls (each dma_start trigger costs ~0.8us serially on
        # the sync queue); single samples first so sigmoid_0 starts early,
        # pairs after, mask (needed only at the end) last.
        wv2 = w_v.rearrange("p b f -> p (b f)")
        for lo, hi in ((0, 1), (1, 2), (2, 4), (4, 6), (6, 8)):
            nc.sync.dma_start(out=wt[:, lo * FREE:hi * FREE],
                              in_=wv2[:, lo * FREE:hi * FREE])
        nc.sync.dma_start(out=mask_t[:], in_=mask_in.ap())

        # ACT phase 1: all sigmoids back-to-back -> one table load.
        for b in range(B_PER_CORE):
            nc.scalar.activation(ut[:, sl(b)], wt[:, sl(b)], Act.Sigmoid)

        # DVE, all fast 16-bit ops: u2 = u^2 (tt); su2 = s*u^2 via sign-bit
        # XOR with s = 1-2t from w's LSB (pure bitwise stt); then
        # F = 0.5*su2 + u2 = (1.5-t)*u^2 (all-bf16 stt).
        wu = wt[:].bitcast(u16)
        fts = [None] * B_PER_CORE

        for b in range(B_PER_CORE):
            u2 = u2pool.tile([P, FREE], bf16, tag="u2")
            nc.vector.tensor_tensor(
                out=u2[:], in0=ut[:, sl(b)], in1=ut[:, sl(b)], op=Alu.mult)
            su2 = s2pool.tile([P, FREE], bf16, tag="su2")
            nc.vector.scalar_tensor_tensor(
                out=su2[:].bitcast(u16), in0=wu[:, sl(b)], scalar=shift15_t[:],
                in1=u2[:].bitcast(u16),
                op0=Alu.logical_shift_left, op1=Alu.bitwise_xor)
            ft = fpool.tile([P, FREE], bf16, tag="ft")
            nc.vector.scalar_tensor_tensor(
                out=ft[:], in0=su2[:], scalar=0.5, in1=u2[:],
                op0=Alu.mult, op1=Alu.add)
            fts[b] = ft

        # ACT phase 2 (Ln, second table load) + PE per sample.
        pss = [None] * B_PER_CORE
        for b in range(B_PER_CORE):
            lv = lvpool.tile([P, FREE], bf16, tag="lv")
            nc.scalar.activation(lv[:], ut[:, sl(b)], Act.Ln,
                                 scale=-1.0, bias=1.0)
            ps = psum.tile([P, P], f32, tag="ps")
            for c in range(NCHUNK):
                cs = slice(c * P, (c + 1) * P)
                nc.tensor.matmul(ps[:], lv[:, cs], fts[b][:, cs],
                                 start=(c == 0), stop=(c == NCHUNK - 1))
            pss[b] = ps

        # Diag extraction (end of DVE queue; each waits on its PE chain).
        for b in range(B_PER_CORE):
            scr = scrpool.tile([P, P], f32, tag="scr")
            nc.vector.scalar_tensor_tensor(
                out=scr[:], in0=pss[b][:], scalar=0.0, in1=mask_t[:],
                op0=Alu.add, op1=Alu.mult,
                accum_out=Scol[:, b:b + 1])

        nc.sync.dma_start(out=out_t.ap(), in_=Scol[:])

    nc.compile()
    return nc


def _get_nc():
    if "nc" not in _GLOBAL:
        _GLOBAL["nc"] = _build()
    return _GLOBAL["nc"]


def _mask_np():
    m = np.zeros((P, P), dtype=np.float32)
    idx = np.arange(P)
    m[idx, idx] = -0.5         # elem = -0.5 * F * Lv, F = (1.5-t)*u^2
    return m


GAMMA = 2.0
ALPHA = 0.25
SIZE_POWER = 0.5


def _pack_w(pred: np.ndarray, target: np.ndarray) -> np.ndarray:
    """w = bf16_rne(pred)*(1-2t) with t in the LSB; [64, P, FREE] bf16."""
    import ml_dtypes

    x = np.ascontiguousarray(pred[:, 0])
    t = (target > 0).astype(np.uint16)
    bits = x.view(np.uint32)
    hi = ((bits + np.uint32(0x7FFF) + ((bits >> np.uint32(16)) & np.uint32(1)))
          >> np.uint32(16)).astype(np.uint16)
    w16 = ((hi ^ (t << np.uint16(15))) & np.uint16(0xFFFE)) | t
    # [B, 512, 512] -> [B, 128, 2048]: row-major (p q) w -> p (q w), no copy
    return w16.reshape(-1, P, FREE).view(ml_dtypes.bfloat16)


def _core_layout(wv_core: np.ndarray) -> np.ndarray:
    """[8, P, FREE] -> [P, 8, FREE] contiguous: per-partition DRAM rows hold
    all samples back to back, so grouped DMAs use large contiguous reads."""
    return np.ascontiguousarray(wv_core.transpose(1, 0, 2))


def kernel(pred: np.ndarray, target: np.ndarray) -> np.ndarray:
    from concourse import bass_utils

    nc = _get_nc()
    pred = np.ascontiguousarray(np.asarray(pred, dtype=np.float32))
    target = np.ascontiguousarray(np.asarray(target, dtype=np.int32))
    wv = _pack_w(pred, target)
    mask = _mask_np()

    in_maps = []
    for i in range(N_CORES):
        s = slice(i * B_PER_CORE, (i + 1) * B_PER_CORE)
        in_maps.append({
            "wp": _core_layout(wv[s]),
            "mask": mask,
        })

    res = bass_utils.run_bass_kernel_spmd(
        nc, in_maps, core_ids=list(range(N_CORES)),
        trace=bool(_GLOBAL.get("trace", False)),
        **_GLOBAL.get("run_kwargs", {}),
    )
    _GLOBAL["last_results"] = res

    outs = np.stack([r["out"] for r in res.results], axis=0)  # [8, 128, 8]
    S = outs.astype(np.float64).sum(axis=1).reshape(-1)       # per-sample sums
    fg = np.count_nonzero(target.reshape(target.shape[0], -1), axis=1)
    fg = fg.astype(np.float64)
    sw = np.where(fg > 0,
                  np.minimum(100.0 / np.power(np.maximum(fg, 1.0), SIZE_POWER), 10.0),
                  1.0)
    per_sample = (S / HW) * sw
    return np.float32(per_sample.mean())


# revision 12
# speedup vs baseline: 1.2485x; 1.2485x over previous
"""Size-weighted focal loss on 8 Trainium2 NeuronCores — v5.

Math (per element, x = logit, t in {0,1}):
  w  = x*(1-2t)         so (1-pt) = sigmoid(w) = u
  L  = -log(pt) = softplus(w) = -ln(1-u)
  a  = 0.75 - 0.5*t     (alpha_t with ALPHA=0.25)
  elem = a * u^2 * L

Host packs w = bf16_rne(x) sign-flipped by t, with t stowed in the bf16
LSB (<=1ulp perturbation). Device input is 4MB/core instead of 16MB —
the baseline's DMA bottleneck — and the strided hi16-XOR DVE pass
disappears.

Device (per core, 8 samples, phase-ordered so each ACT table loads once):
  u    = Sigmoid(w)            [ACT pass 1, table sigmoid_and_others]
  Lv   = Ln(1 - u)  = -L       [ACT pass 2, table natural_log]
  tm   = (w&1) - 1.5 = t - 1.5 [DVE ts, int AND then float SUB]
  g    = tm * u                [DVE tt]
  F    = g * u = (t-1.5)*u^2   [DVE tt]
  PE per sample: psum[128,128] += Lv_chunk^T @ F_chunk  (16 chunks)
  diag extract with mask M[i,i] = 0.5:
    Scol[:,b] = 0.5*sum_diag = sum(a*u^2*L) partials per partition-slot
  (elem = a*u^2*L = 0.5*F*Lv since a = -0.5*(t-1.5), L = -Lv)

Host: fg_b = count_nonzero(target_b); mean_b( (S_b/HW) * sw(fg_b) ).
"""

import numpy as np
from contextlib import ExitStack

P = 128
B_PER_CORE = 8
N_CORES = 8
H = 512
W = 512
HW = H * W                 # 262144
FREE = HW // P             # 2048 per sample
NCHUNK = FREE // P         # 16 chunks per sample

_GLOBAL = {}


def _build():
    import concourse.bacc as bacc
    import concourse.tile as tile
    import concourse.mybir as mybir

    f32 = mybir.dt.float32
    bf16 = mybir.dt.bfloat16
    u16 = mybir.dt.uint16
    Alu = mybir.AluOpType
    Act = mybir.ActivationFunctionType

    nc = bacc.Bacc("TRN2", target_bir_lowering=False, debug=False,
                   num_devices=N_CORES)

    wp_in = nc.dram_tensor("wp", (P, B_PER_CORE, FREE), bf16, kind="ExternalInput")
    mask_in = nc.dram_tensor("mask", (P, P), f32, kind="ExternalInput")
    out_t = nc.dram_tensor("out", (P, B_PER_CORE), f32, kind="ExternalOutput")

    w_v = wp_in.ap()

    with ExitStack() as ctx:
        tc = ctx.enter_context(tile.TileContext(nc))
        singles = ctx.enter_context(tc.tile_pool(name="singles", bufs=1))
        u2pool = ctx.enter_context(tc.tile_pool(name="u2pool", bufs=3))
        s2pool = ctx.enter_context(tc.tile_pool(name="s2pool", bufs=2))
        fpool = ctx.enter_context(tc.tile_pool(name="fpool", bufs=8))
        lvpool = ctx.enter_context(tc.tile_pool(name="lvpool", bufs=3))
        scrpool = ctx.enter_context(tc.tile_pool(name="scrpool", bufs=2))
        psum = ctx.enter_context(tc.tile_pool(name="psum", bufs=8, space="PSUM"))

        mask_t = singles.tile([P, P], f32)
        Scol = singles.tile([P, B_PER_CORE], f32)
        wt = singles.tile([P, B_PER_CORE * FREE], bf16)   # packed w, all samples
        ut = singles.tile([P, B_PER_CORE * FREE], bf16)   # sigmoid(w)

        def sl(b):
            return slice(b * FREE, (b + 1) * FREE)

        # DMA: few calls (fewer sync-queue instructions); sample 0 split in
        # quarters so sigmoid_0 starts as soon as the rings come up, sample
        # 1 in halves, pairs after, mask (needed only at the end) last.
        wv2 = w_v.rearrange("p b f -> p (b f)")
        Q = FREE // 4
        dma_cuts = [0, Q, 2 * Q, 3 * Q, FREE, FREE + FREE // 2,
                    2 * FREE, 4 * FREE, 6 * FREE, 8 * FREE]
        for lo, hi in zip(dma_cuts[:-1], dma_cuts[1:]):
            nc.sync.dma_start(out=wt[:, lo:hi], in_=wv2[:, lo:hi])
        nc.sync.dma_start(out=mask_t[:], in_=mask_in.ap())

        # ACT phase 1: all sigmoids back-to-back -> one table load.
        # Sample 0 in quarters to chase its quarter-DMAs.
        for lo, hi in zip(dma_cuts[:4], dma_cuts[1:5]):
            nc.scalar.activation(ut[:, lo:hi], wt[:, lo:hi], Act.Sigmoid)
        for b in range(1, B_PER_CORE):
            nc.scalar.activation(ut[:, sl(b)], wt[:, sl(b)], Act.Sigmoid)

        # DVE: tm = t (u16, cheap 2-op ts); g = (tm-1.5)*u (stt);
        # F = g*u = (t-1.5)*u^2 (tt).
        wu = wt[:].bitcast(u16)
        fts = [None] * B_PER_CORE

        for b in range(B_PER_CORE):
            tm = s2pool.tile([P, FREE], u16, tag="tm")
            nc.vector.tensor_scalar(
                out=tm[:], in0=wu[:, sl(b)], scalar1=1, scalar2=0,
                op0=Alu.bitwise_and, op1=Alu.bitwise_or)
            g = u2pool.tile([P, FREE], bf16, tag="g")
            nc.vector.scalar_tensor_tensor(
                out=g[:], in0=tm[:], scalar=1.5, in1=ut[:, sl(b)],
                op0=Alu.subtract, op1=Alu.mult)
            ft = fpool.tile([P, FREE], bf16, tag="ft")
            nc.vector.tensor_tensor(
                out=ft[:], in0=g[:], in1=ut[:, sl(b)], op=Alu.mult)
            fts[b] = ft

        # ACT phase 2 (Ln, second table load) + PE per sample.
        pss = [None] * B_PER_CORE
        for b in range(B_PER_CORE):
            lv = lvpool.tile([P, FREE], bf16, tag="lv")
            nc.scalar.activation(lv[:], ut[:, sl(b)], Act.Ln,
                                 scale=-1.0, bias=1.0)
            ps = psum.tile([P, P], f32, tag="ps")
            for c in range(NCHUNK):
                cs = slice(c * P, (c + 1) * P)
                nc.tensor.matmul(ps[:], lv[:, cs], fts[b][:, cs],
                                 start=(c == 0), stop=(c == NCHUNK - 1))
            pss[b] = ps

        # Diag extraction (end of DVE queue; each waits on its PE chain).
        for b in range(B_PER_CORE):
            scr = scrpool.tile([P, P], f32, tag="scr")
            nc.vector.scalar_tensor_tensor(
                out=scr[:], in0=pss[b][:], scalar=0.0, in1=mask_t[:],
                op0=Alu.add, op1=Alu.mult,
                accum_out=Scol[:, b:b + 1])

        nc.sync.dma_start(out=out_t.ap(), in_=Scol[:])

    nc.compile()
    return nc


def _get_nc():
    if "nc" not in _GLOBAL:
        _GLOBAL["nc"] = _build()
    return _GLOBAL["nc"]


def _mask_np():
    m = np.zeros((P, P), dtype=np.float32)
    idx = np.arange(P)
    m[idx, idx] = 0.5          # elem = 0.5 * F * Lv, F = (t-1.5)*u^2
    return m


GAMMA = 2.0
ALPHA = 0.25
SIZE_POWER = 0.5


def _pack_w(pred: np.ndarray, target: np.ndarray) -> np.ndarray:
    """w = bf16_rne(pred)*(1-2t) with t in the LSB; [64, P, FREE] bf16."""
    import ml_dtypes

    x = np.ascontiguousarray(pred[:, 0])
    t = (target > 0).astype(np.uint16)
    bits = x.view(np.uint32)
    hi = ((bits + np.uint32(0x7FFF) + ((bits >> np.uint32(16)) & np.uint32(1)))
          >> np.uint32(16)).astype(np.uint16)
    w16 = ((hi ^ (t << np.uint16(15))) & np.uint16(0xFFFE)) | t
    # [B, 512, 512] -> [B, 128, 2048]: row-major (p q) w -> p (q w), no copy
    return w16.reshape(-1, P, FREE).view(ml_dtypes.bfloat16)


def _core_layout(wv_core: np.ndarray) -> np.ndarray:
    """[8, P, FREE] -> [P, 8, FREE] contiguous: per-partition DRAM rows hold
    all samples back to back, so grouped DMAs use large contiguous reads."""
    return np.ascontiguousarray(wv_core.transpose(1, 0, 2))


def kernel(pred: np.ndarray, target: np.ndarray) -> np.ndarray:
    from concourse import bass_utils

    nc = _get_nc()
    pred = np.ascontiguousarray(np.asarray(pred, dtype=np.float32))
    target = np.ascontiguousarray(np.asarray(target, dtype=np.int32))
    wv = _pack_w(pred, target)
    mask = _mask_np()

    in_maps = []
    for i in range(N_CORES):
        s = slice(i * B_PER_CORE, (i + 1) * B_PER_CORE)
        in_maps.append({
            "wp": _core_layout(wv[s]),
            "mask": mask,
        })

    res = bass_utils.run_bass_kernel_spmd(
        nc, in_maps, core_ids=list(range(N_CORES)),
        trace=bool(_GLOBAL.get("trace", False)),
        **_GLOBAL.get("run_kwargs", {}),
    )
    _GLOBAL["last_results"] = res

    outs = np.stack([r["out"] for r in res.results], axis=0)  # [8, 128, 8]
    S = outs.astype(np.float64).sum(axis=1).reshape(-1)       # per-sample sums
    fg = np.count_nonzero(target.reshape(target.shape[0], -1), axis=1)
    fg = fg.astype(np.float64)
    sw = np.where(fg > 0,
                  np.minimum(100.0 / np.power(np.maximum(fg, 1.0), SIZE_POWER), 10.0),
                  1.0)
    per_sample = (S / HW) * sw
    return np.float32(per_sample.mean())


# revision 14
# speedup vs baseline: 1.5050x; 1.2054x over previous
"""Size-weighted focal loss on 8 Trainium2 NeuronCores — v5.

Math (per element, x = logit, t in {0,1}):
  w  = x*(1-2t)         so (1-pt) = sigmoid(w) = u
  L  = -log(pt) = softplus(w) = -ln(1-u)
  a  = 0.75 - 0.5*t     (alpha_t with ALPHA=0.25)
  elem = a * u^2 * L

Host packs w = bf16_rne(x) sign-flipped by t, with t stowed in the bf16
LSB (<=1ulp perturbation). Device input is 4MB/core instead of 16MB —
the baseline's DMA bottleneck — and the strided hi16-XOR DVE pass
disappears.

Device (per core, 8 samples, phase-ordered so each ACT table loads once):
  u    = Sigmoid(w)            [ACT pass 1, table sigmoid_and_others]
  Lv   = Ln(1 - u)  = -L       [ACT pass 2, table natural_log]
  tm   = (w&1) - 1.5 = t - 1.5 [DVE ts, int AND then float SUB]
  g    = tm * u                [DVE tt]
  F    = g * u = (t-1.5)*u^2   [DVE tt]
  PE per sample: psum[128,128] += Lv_chunk^T @ F_chunk  (16 chunks)
  diag extract with mask M[i,i] = 0.5:
    Scol[:,b] = 0.5*sum_diag = sum(a*u^2*L) partials per partition-slot
  (elem = a*u^2*L = 0.5*F*Lv since a = -0.5*(t-1.5), L = -Lv)

Host: fg_b = count_nonzero(target_b); mean_b( (S_b/HW) * sw(fg_b) ).
"""

import numpy as np
from contextlib import ExitStack

P = 128
B_PER_CORE = 8
N_CORES = 8
H = 512
W = 512
HW = H * W                 # 262144
FREE = HW // P             # 2048 per sample
NCHUNK = FREE // P         # 16 chunks per sample

_GLOBAL = {}


def _build():
    import concourse.bacc as bacc
    import concourse.tile as tile
    import concourse.mybir as mybir

    f32 = mybir.dt.float32
    bf16 = mybir.dt.bfloat16
    u16 = mybir.dt.uint16
    Alu = mybir.AluOpType
    Act = mybir.ActivationFunctionType

    nc = bacc.Bacc("TRN2", target_bir_lowering=False, debug=False,
                   num_devices=N_CORES)

    wp_in = nc.dram_tensor("wp", (P, B_PER_CORE, FREE), bf16, kind="ExternalInput")
    mask_in = nc.dram_tensor("mask", (P, P), f32, kind="ExternalInput")
    out_t = nc.dram_tensor("out", (P, B_PER_CORE), f32, kind="ExternalOutput")

    w_v = wp_in.ap()

    with ExitStack() as ctx:
        tc = ctx.enter_context(tile.TileContext(nc))
        singles = ctx.enter_context(tc.tile_pool(name="singles", bufs=1))
        u2pool = ctx.enter_context(tc.tile_pool(name="u2pool", bufs=3))
        s2pool = ctx.enter_context(tc.tile_pool(name="s2pool", bufs=2))
        fpool = ctx.enter_context(tc.tile_pool(name="fpool", bufs=8))
        lvpool = ctx.enter_context(tc.tile_pool(name="lvpool", bufs=3))
        scrpool = ctx.enter_context(tc.tile_pool(name="scrpool", bufs=2))
        psum = ctx.enter_context(tc.tile_pool(name="psum", bufs=8, space="PSUM"))

        mask_t = singles.tile([P, P], f32)
        Scol = singles.tile([P, B_PER_CORE], f32)
        wt = singles.tile([P, B_PER_CORE * FREE], bf16)   # packed w, all samples
        ut = singles.tile([P, B_PER_CORE * FREE], bf16)   # sigmoid(w)

        def sl(b):
            return slice(b * FREE, (b + 1) * FREE)

        # DMA: few calls (fewer sync-queue instructions); sample 0 split in
        # quarters so sigmoid_0 starts as soon as the rings come up, sample
        # 1 in halves, pairs after, mask (needed only at the end) last.
        wv2 = w_v.rearrange("p b f -> p (b f)")
        Hh = FREE // 2
        dma_cuts = [0, Hh, FREE, 2 * FREE, 4 * FREE, 6 * FREE, 8 * FREE]
        for lo, hi in zip(dma_cuts[:-1], dma_cuts[1:]):
            nc.sync.dma_start(out=wt[:, lo:hi], in_=wv2[:, lo:hi])
        nc.sync.dma_start(out=mask_t[:], in_=mask_in.ap())

        # ACT phase 1: all sigmoids back-to-back -> one table load.
        # Sample 0 in halves to chase its half-DMAs.
        sig_last = None
        for lo, hi in ((0, Hh), (Hh, FREE)):
            sig_last = nc.scalar.activation(ut[:, lo:hi], wt[:, lo:hi],
                                            Act.Sigmoid)
        for b in range(1, B_PER_CORE):
            sig_last = nc.scalar.activation(ut[:, sl(b)], wt[:, sl(b)],
                                            Act.Sigmoid)

        # DVE: tm = t (u16, cheap 2-op ts); g = (tm-1.5)*u (stt);
        # F = g*u = (t-1.5)*u^2 (tt).
        wu = wt[:].bitcast(u16)
        fts = [None] * B_PER_CORE

        for b in range(B_PER_CORE):
            tm = s2pool.tile([P, FREE], u16, tag="tm")
            nc.vector.tensor_scalar(
                out=tm[:], in0=wu[:, sl(b)], scalar1=1, scalar2=0,
                op0=Alu.bitwise_and, op1=Alu.bitwise_or)
            g = u2pool.tile([P, FREE], bf16, tag="g")
            nc.vector.scalar_tensor_tensor(
                out=g[:], in0=tm[:], scalar=1.5, in1=ut[:, sl(b)],
                op0=Alu.subtract, op1=Alu.mult)
            ft = fpool.tile([P, FREE], bf16, tag="ft")
            nc.vector.tensor_tensor(
                out=ft[:], in0=g[:], in1=ut[:, sl(b)], op=Alu.mult)
            fts[b] = ft

        # ACT phase 2 (Ln, second table load) + PE per sample. The first Ln
        # gets an artificial dep on the last sigmoid so the ACT list
        # scheduler cannot interleave the phases (which would thrash the
        # activation tables: sigmoid and ln live in different sets).
        import bass_rust as _br
        pss = [None] * B_PER_CORE
        for b in range(B_PER_CORE):
            lv = lvpool.tile([P, FREE], bf16, tag="lv")
            ln_inst = nc.scalar.activation(lv[:], ut[:, sl(b)], Act.Ln,
                                           scale=-1.0, bias=1.0)
            if b == 0:
                ln_inst.ins.add_nosync_dependencies_from(
                    _br.InstructionNameOrderedSet([sig_last.ins.name]))
            ps = psum.tile([P, P], f32, tag="ps")
            for c in range(NCHUNK):
                cs = slice(c * P, (c + 1) * P)
                nc.tensor.matmul(ps[:], lv[:, cs], fts[b][:, cs],
                                 start=(c == 0), stop=(c == NCHUNK - 1))
            pss[b] = ps

        # Diag extraction (end of DVE queue; each waits on its PE chain).
        for b in range(B_PER_CORE):
            scr = scrpool.tile([P, P], f32, tag="scr")
            nc.vector.scalar_tensor_tensor(
                out=scr[:], in0=pss[b][:], scalar=0.0, in1=mask_t[:],
                op0=Alu.add, op1=Alu.mult,
                accum_out=Scol[:, b:b + 1])

        nc.sync.dma_start(out=out_t.ap(), in_=Scol[:])

    nc.compile()
    return nc


def _get_nc():
    if "nc" not in _GLOBAL:
        _GLOBAL["nc"] = _build()
    return _GLOBAL["nc"]


def _mask_np():
    m = np.zeros((P, P), dtype=np.float32)
    idx = np.arange(P)
    m[idx, idx] = 0.5          # elem = 0.5 * F * Lv, F = (t-1.5)*u^2
    return m


GAMMA = 2.0
ALPHA = 0.25
SIZE_POWER = 0.5


def _pack_w(pred: np.ndarray, target: np.ndarray) -> np.ndarray:
    """w = bf16_rne(pred)*(1-2t) with t in the LSB; [64, P, FREE] bf16."""
    import ml_dtypes

    x = np.ascontiguousarray(pred[:, 0])
    t = (target > 0).astype(np.uint16)
    bits = x.view(np.uint32)
    hi = ((bits + np.uint32(0x7FFF) + ((bits >> np.uint32(16)) & np.uint32(1)))
          >> np.uint32(16)).astype(np.uint16)
    w16 = ((hi ^ (t << np.uint16(15))) & np.uint16(0xFFFE)) | t
    # [B, 512, 512] -> [B, 128, 2048]: row-major (p q) w -> p (q w), no copy
    return w16.reshape(-1, P, FREE).view(ml_dtypes.bfloat16)


def _core_layout(wv_core: np.ndarray) -> np.ndarray:
    """[8, P, FREE] -> [P, 8, FREE] contiguous: per-partition DRAM rows hold
    all samples back to back, so grouped DMAs use large contiguous reads."""
    return np.ascontiguousarray(wv_core.transpose(1, 0, 2))


def kernel(pred: np.ndarray, target: np.ndarray) -> np.ndarray:
    from concourse import bass_utils

    nc = _get_nc()
    pred = np.ascontiguousarray(np.asarray(pred, dtype=np.float32))
    target = np.ascontiguousarray(np.asarray(target, dtype=np.int32))
    wv = _pack_w(pred, target)
    mask = _mask_np()

    in_maps = []
    for i in range(N_CORES):
        s = slice(i * B_PER_CORE, (i + 1) * B_PER_CORE)
        in_maps.append({
            "wp": _core_layout(wv[s]),
            "mask": mask,
        })

    res = bass_utils.run_bass_kernel_spmd(
        nc, in_maps, core_ids=list(range(N_CORES)),
        trace=bool(_GLOBAL.get("trace", False)),
        **_GLOBAL.get("run_kwargs", {}),
    )
    _GLOBAL["last_results"] = res

    outs = np.stack([r["out"] for r in res.results], axis=0)  # [8, 128, 8]
    S = outs.astype(np.float64).sum(axis=1).reshape(-1)       # per-sample sums
    fg = np.count_nonzero(target.reshape(target.shape[0], -1), axis=1)
    fg = fg.astype(np.float64)
    sw = np.where(fg > 0,
                  np.minimum(100.0 / np.power(np.maximum(fg, 1.0), SIZE_POWER), 10.0),
                  1.0)
    per_sample = (S / HW) * sw
    return np.float32(per_sample.mean())
